# revision 3
# baseline (speedup 1.0000x reference)
"""AlignSnippets Trainium2 kernel (nn_AlignSnippets_48198122996101).

Data-parallel over 8 NeuronCores: the event dimension N=100000 is split into
8 slices of R=12500 rows; the tiny [K=64] offsets table is replicated.

Per-core layout: R rows as [P=125 partitions x F=100 cols], row n = p*F + j.
The per-row roll of each [T=64, C=16] snippet by shift s = offsets[label-1]
(s in [0,16)) is done with two indirect-DMA gathers at 64-byte chunk
granularity (one column of 125 rows per call, one index per partition):
  g1: 64-chunk window at chunk offset n*64 - s   -> chunks [s,63] correct
  g2: 15-chunk window at chunk offset n*64+64-s  -> head chunks land at [0,s)
one copy_predicated merges g2's head over g1 where chunk_pos < s, and a
multiply by valid zeroes rows with label outside [1,K]. Rows 0 and R-1 of
each core need wrap data the clamped windows cannot reach; they are rebuilt
from a wrap-extended DRAM scratch copy and overwritten at the end.
times2 = (times - s) * valid is computed with exact 16-bit limb arithmetic
(the DVE ALU is fp32 internally; times exceed 2^24).
"""

import sys

for _p in ("/opt/trn_rl_repo", "/opt/pypackages"):
    if _p not in sys.path:
        sys.path.append(_p)

from contextlib import ExitStack

import numpy as np

import concourse.bass as bass
import concourse.bacc as bacc
import concourse.mybir as mybir
import concourse.tile as tile
from concourse.tile_rust import add_dep_helper
from concourse.bass_utils import run_bass_kernel_spmd

dt = mybir.dt
Alu = mybir.AluOpType

N = 100000      # events
T = 64          # snippet length (chunks per row)
CH = 16         # channels (elements per chunk)
K = 64          # number of clusters
N_CORES = 8
R = N // N_CORES    # rows per core (12500)
P = 125             # partitions used
F = R // P          # columns (100)
NBUFS = 8


def _build():
    VCH = R * T          # total chunks in this core's snippet slice
    HCH = 15             # head window chunks
    HH = HCH * CH        # head window elements (240)
    RW = T * CH          # row elements (1024)
    ECH = HCH + T        # 79 chunks per wrap-extended edge row

    nc = bacc.Bacc("TRN2", target_bir_lowering=False, debug=False)

    snip = nc.dram_tensor("snip", [VCH, CH], dt.float32, kind="ExternalInput")
    lab = nc.dram_tensor("labels_i32", [P, 2 * F], dt.int32, kind="ExternalInput")
    tim = nc.dram_tensor("times_i32", [P, 2 * F], dt.int32, kind="ExternalInput")
    offs = nc.dram_tensor("offsets", [1, K], dt.int32, kind="ExternalInput")
    outt = nc.dram_tensor("aligned", [VCH, CH], dt.float32, kind="ExternalOutput")
    t2 = nc.dram_tensor("times2_i32", [P, 2 * F], dt.int32, kind="ExternalOutput")
    escr = nc.dram_tensor("edge_scratch", [2 * ECH, CH], dt.float32)

    with ExitStack() as ctx:
        tc = ctx.enter_context(tile.TileContext(nc))
        const = ctx.enter_context(tc.tile_pool(name="const", bufs=1))
        work = ctx.enter_context(tc.tile_pool(name="work", bufs=NBUFS))

        # ---- labels ----
        lab_sb = const.tile([P, 2 * F], dt.int32)
        nc.sync.dma_start(lab_sb[:], lab[:, :])
        lab_lo = const.tile([P, F], dt.int32)
        nc.vector.tensor_copy(lab_lo[:], lab_sb[:, 0 : 2 * F : 2])

        # invalid = (label < 1) | (label > K); invm1 = invalid - 1 (0 or -1)
        invalid = const.tile([P, F], dt.int32)
        tmpv = const.tile([P, F], dt.int32)
        nc.vector.tensor_scalar(
            out=invalid[:], in0=lab_lo[:], scalar1=1, scalar2=None, op0=Alu.is_lt
        )
        nc.vector.tensor_scalar(
            out=tmpv[:], in0=lab_lo[:], scalar1=K, scalar2=None, op0=Alu.is_gt
        )
        nc.vector.tensor_tensor(
            out=invalid[:], in0=invalid[:], in1=tmpv[:], op=Alu.add
        )
        invm1 = const.tile([P, F], dt.int32)
        nc.vector.tensor_scalar(
            out=invm1[:], in0=invalid[:], scalar1=1, scalar2=None, op0=Alu.subtract
        )
        # validf = 1 - invalid, as float for per-partition scalar multiplies
        validf = const.tile([P, F], dt.float32)
        nc.vector.tensor_scalar(
            out=validf[:], in0=invalid[:], scalar1=-1, scalar2=1,
            op0=Alu.mult, op1=Alu.add,
        )

        # ---- shift = offsets[label-1] (0 if invalid) via compare-accumulate
        # over the K table entries, offsets replicated to all partitions ----
        offs_b = const.tile([P, K], dt.int32)
        nc.sync.dma_start(offs_b[:], offs.ap().to_broadcast([P, K]))
        shift = const.tile([P, F], dt.int32)
        nc.gpsimd.memset(shift[:], 0)
        mtmp = const.tile([P, F], dt.int32)
        for k in range(K):
            nc.vector.scalar_tensor_tensor(
                out=mtmp[:], in0=lab_lo[:], scalar=float(k + 1),
                in1=offs_b[:, k : k + 1].to_broadcast([P, F]),
                op0=Alu.is_equal, op1=Alu.mult,
            )
            nc.vector.tensor_tensor(
                out=shift[:], in0=shift[:], in1=mtmp[:], op=Alu.add
            )

        # ---- gather index vectors ----
        base64 = const.tile([P, F], dt.int32)
        nc.gpsimd.iota(
            base64[:], pattern=[[T, F]], base=0, channel_multiplier=T * F
        )
        bm = const.tile([P, F], dt.int32)
        nc.vector.tensor_tensor(out=bm[:], in0=base64[:], in1=shift[:], op=Alu.subtract)
        idx1 = const.tile([P, F], dt.int32)
        nc.vector.tensor_scalar(
            out=idx1[:], in0=bm[:], scalar1=0, scalar2=None, op0=Alu.max
        )
        idx2 = const.tile([P, F], dt.int32)
        nc.vector.tensor_scalar(
            out=idx2[:], in0=bm[:], scalar1=T, scalar2=VCH - HCH, op0=Alu.add,
            op1=Alu.min,
        )

        # chunk-position iota for the head region (values 0..14 per chunk)
        cpos = const.tile([P, HH], dt.int32)
        nc.gpsimd.iota(
            cpos[:], pattern=[[1, HCH], [0, CH]], base=0, channel_multiplier=0
        )

        # ---- times2 = (times - shift) * valid, exact via 16-bit limbs ----
        tim_sb = const.tile([P, 2 * F], dt.int32)
        nc.sync.dma_start(tim_sb[:], tim[:, :])
        tlo = tim_sb[:, 0 : 2 * F : 2]
        a16 = const.tile([P, F], dt.int32)
        nc.vector.tensor_scalar(
            out=a16[:], in0=tlo, scalar1=0xFFFF, scalar2=None, op0=Alu.bitwise_and
        )
        b16 = const.tile([P, F], dt.int32)
        nc.vector.tensor_scalar(
            out=b16[:], in0=tlo, scalar1=16, scalar2=None, op0=Alu.logical_shift_right
        )
        d16 = const.tile([P, F], dt.int32)
        nc.vector.tensor_tensor(out=d16[:], in0=a16[:], in1=shift[:], op=Alu.subtract)
        borrow = const.tile([P, F], dt.int32)
        nc.vector.tensor_scalar(
            out=borrow[:], in0=d16[:], scalar1=0, scalar2=None, op0=Alu.is_lt
        )
        loc = const.tile([P, F], dt.int32)
        nc.vector.scalar_tensor_tensor(
            out=loc[:], in0=borrow[:], scalar=65536, in1=d16[:],
            op0=Alu.mult, op1=Alu.add,
        )
        hic = const.tile([P, F], dt.int32)
        nc.vector.tensor_tensor(out=hic[:], in0=b16[:], in1=borrow[:], op=Alu.subtract)
        nc.vector.tensor_scalar(
            out=hic[:], in0=hic[:], scalar1=16, scalar2=None, op0=Alu.arith_shift_left
        )
        low32 = const.tile([P, F], dt.int32)
        nc.vector.tensor_tensor(out=low32[:], in0=hic[:], in1=loc[:], op=Alu.bitwise_or)
        tout_sb = const.tile([P, 2 * F], dt.int32)
        nc.vector.tensor_tensor(
            out=tout_sb[:, 0 : 2 * F : 2], in0=low32[:], in1=invm1[:],
            op=Alu.bitwise_and,
        )
        nc.vector.tensor_scalar(
            out=tout_sb[:, 1 : 2 * F : 2], in0=tout_sb[:, 0 : 2 * F : 2],
            scalar1=31, scalar2=None, op0=Alu.arith_shift_right,
        )
        nc.sync.dma_start(t2[:, :], tout_sb[:])

        # ---- main loop: one column (125 rows) per iteration ----
        out_flat = outt.ap().rearrange("(p f) c -> p (f c)", p=P)
        col_dmas = []
        for j in range(F):
            raw = work.tile([P, RW], dt.float32)
            nc.gpsimd.indirect_dma_start(
                out=raw[:],
                out_offset=None,
                in_=snip[:, :],
                in_offset=bass.IndirectOffsetOnAxis(ap=idx1[:, j : j + 1], axis=0),
            )
            head = work.tile([P, HH], dt.float32)
            nc.gpsimd.indirect_dma_start(
                out=head[:],
                out_offset=None,
                in_=snip[:, :],
                in_offset=bass.IndirectOffsetOnAxis(ap=idx2[:, j : j + 1], axis=0),
            )
            mask = work.tile([P, HH], dt.int32)
            nc.vector.tensor_tensor(
                out=mask[:],
                in0=cpos[:],
                in1=shift[:, j : j + 1].to_broadcast([P, HH]),
                op=Alu.is_lt,
            )
            nc.vector.copy_predicated(out=raw[:, 0:HH], mask=mask[:], data=head[:])
            nc.vector.tensor_scalar(
                out=raw[:], in0=raw[:], scalar1=validf[:, j : j + 1], scalar2=None,
                op0=Alu.mult,
            )
            col_dmas.append(
                nc.sync.dma_start(out=out_flat[:, j * RW : (j + 1) * RW], in_=raw[:])
            )

        # ---- edge fix: rows 0 and R-1 ----
        snip_ap = snip.ap()
        HCHW = HCH
        ew = [
            nc.sync.dma_start(escr[0:HCHW, :], snip_ap[T - HCHW : T, :]),
            nc.sync.dma_start(escr[HCHW:ECH, :], snip_ap[0:T, :]),
            nc.sync.dma_start(
                escr[ECH : ECH + HCHW, :], snip_ap[VCH - HCHW : VCH, :]
            ),
            nc.sync.dma_start(
                escr[ECH + HCHW : 2 * ECH, :], snip_ap[VCH - T : VCH, :]
            ),
        ]
        es = const.tile([2, 1], dt.int32)
        nc.sync.dma_start(es[0:1, :], shift[0:1, 0:1])
        nc.sync.dma_start(es[1:2, :], shift[P - 1 : P, F - 1 : F])
        evf = const.tile([2, 1], dt.float32)
        nc.sync.dma_start(evf[0:1, :], validf[0:1, 0:1])
        nc.sync.dma_start(evf[1:2, :], validf[P - 1 : P, F - 1 : F])
        # eidx[p, u] = p*79 + 15 + 32*u - s_p   (u in {0,1})
        eidx = const.tile([2, 2], dt.int32)
        nc.gpsimd.iota(eidx[:], pattern=[[32, 2]], base=HCH, channel_multiplier=ECH)
        nc.vector.tensor_tensor(
            out=eidx[:], in0=eidx[:], in1=es[:].to_broadcast([2, 2]), op=Alu.subtract
        )
        erow = const.tile([2, RW], dt.float32)
        for u in range(2):
            ef = nc.gpsimd.indirect_dma_start(
                out=erow[:, u * 512 : (u + 1) * 512],
                out_offset=None,
                in_=escr[:, :],
                in_offset=bass.IndirectOffsetOnAxis(ap=eidx[:, u : u + 1], axis=0),
            )
            for w in ew:
                add_dep_helper(ef.ins, w.ins, reason="edge scratch RAW")
        nc.vector.tensor_scalar(
            out=erow[:], in0=erow[:], scalar1=evf[:], scalar2=None, op0=Alu.mult
        )
        d1 = nc.sync.dma_start(outt[0:T, :], erow[0:1, :])
        d2 = nc.sync.dma_start(outt[VCH - T : VCH, :], erow[1:2, :])
        add_dep_helper(d1.ins, col_dmas[0].ins, reason="edge row0 after col0")
        add_dep_helper(d2.ins, col_dmas[-1].ins, reason="edge rowlast after collast")

    nc.finalize()
    return nc


_NC = None


def _get_nc():
    global _NC
    if _NC is None:
        _NC = _build()
    return _NC


def _shard_inputs(snippets, times, labels, offsets):
    in_maps = []
    offs32 = np.ascontiguousarray(offsets.astype(np.int32)).reshape(1, K)
    snippets = np.asarray(snippets, dtype=np.float32)
    times = np.asarray(times, dtype=np.int64)
    labels = np.asarray(labels, dtype=np.int64)
    for c in range(N_CORES):
        sl = slice(c * R, (c + 1) * R)
        in_maps.append(
            {
                "snip": np.ascontiguousarray(snippets[sl]).reshape(R * T, CH),
                "labels_i32": np.ascontiguousarray(labels[sl])
                .view(np.int32)
                .reshape(P, 2 * F),
                "times_i32": np.ascontiguousarray(times[sl])
                .view(np.int32)
                .reshape(P, 2 * F),
                "offsets": offs32,
            }
        )
    return in_maps


def _run(snippets, times, labels, offsets, trace=False):
    nc = _get_nc()
    in_maps = _shard_inputs(snippets, times, labels, offsets)
    res = run_bass_kernel_spmd(
        nc, in_maps, core_ids=list(range(N_CORES)), trace=trace
    )
    aligned = np.concatenate(
        [res.results[c]["aligned"].reshape(R, T, CH) for c in range(N_CORES)],
        axis=0,
    )
    times2 = np.concatenate(
        [
            res.results[c]["times2_i32"].reshape(R, 2).view(np.int64).reshape(R)
            for c in range(N_CORES)
        ],
        axis=0,
    )
    return (aligned, times2), res


def kernel(snippets, times, labels, offsets):
    (aligned, times2), _ = _run(snippets, times, labels, offsets)
    return aligned, times2
